# revision 49
# baseline (speedup 1.0000x reference)
"""GATv2 GNN (Graphormer-style) on 8 trn2 NeuronCores.

Strategy: nodes are re-numbered via LPT bin-packing so every 128-node dst
block has <= KT*128 incoming edges (KT=17, near-zero padding) and every core
gets an equal edge count. Per layer: each core computes xl/xr for its shard
(bf16 matmuls), AllGathers xl (bf16), then streams edge tiles: batched
indirect-DMA gathers fetch z = xl[src] (+ xr[dst] fused via compute_op=add),
leaky-relu on GpSimd, per-head logits via one Vector 3D-reduce, Exp on
Scalar, exp-weighted messages scattered into PSUM with one bf16 matmul
against a host-precomputed one-hot mask. Softmax max-subtraction is skipped
(logits are O(10); exact in fp32 since a per-dst constant cancels).
Aggregation uses sum(alpha*z) - xr[d] (valid since sum(alpha)=1), so xl
never needs a second gather. h stays SBUF-resident across layers.
"""
import sys
import numpy as np

sys.path.insert(0, '/opt/trn_rl_repo')

N, E, IN_C, HID, HEADS, L, G, NCLS, MAXDEG = 50000, 800000, 128, 256, 4, 2, 64, 10, 10
HD = HID // HEADS
NCORES = 8
NB = 49                     # dst blocks per core
SHP = NB * 128              # 6272 node slots per core
NTOT = NCORES * SHP
NEG = 0.2
GBZ = 32                    # tiles per batched gather
GBM = 32                    # tiles per batched mask load
CHB = [0, 13, 25, 37, 49]   # block boundaries of AllGather chunks

_CACHE = {}


def _chunk_remap(core, loc):
    """Map (core, local row) -> row in chunk-major xl_full2 layout."""
    block = loc // 128
    chunk = np.digitize(block, CHB[1:4])
    start = np.array(CHB, dtype=np.int64)[chunk] * 128
    rows = (np.array(CHB[1:], dtype=np.int64) -
            np.array(CHB[:-1], dtype=np.int64))[chunk] * 128
    off = np.array([0] + list(np.cumsum(
        [(CHB[i + 1] - CHB[i]) * 128 * NCORES for i in range(4)])[:3]),
        dtype=np.int64)[chunk]
    return off + core * rows + (loc - start)


def _host_prep(edge_index, batch, deg, deg_emb, b_in, x):
    import heapq
    from ml_dtypes import bfloat16

    src0 = np.asarray(edge_index[0], dtype=np.int64)
    dst0 = np.asarray(edge_index[1], dtype=np.int64)
    loops = np.arange(N, dtype=np.int64)
    src_all = np.concatenate([src0, loops])
    dst_all = np.concatenate([dst0, loops])

    indeg = np.bincount(dst_all, minlength=N)
    NBINS = NCORES * NB
    CAP = 128

    # LPT: heaviest nodes first into the lightest (by edge count) non-full bin
    order = np.argsort(-indeg, kind='stable')
    bin_cnt = np.zeros(NBINS, dtype=np.int64)
    bin_n = np.zeros(NBINS, dtype=np.int64)
    bin_of = np.zeros(N, dtype=np.int32)
    slot_of = np.zeros(N, dtype=np.int32)
    heap = [(0, b) for b in range(NBINS)]
    heapq.heapify(heap)
    for i in order:
        while True:
            c, b = heapq.heappop(heap)
            if c != bin_cnt[b] or bin_n[b] >= CAP:
                continue
            break
        bin_of[i] = b
        slot_of[i] = bin_n[b]
        bin_n[b] += 1
        bin_cnt[b] += indeg[i]
        if bin_n[b] < CAP:
            heapq.heappush(heap, (int(bin_cnt[b]), b))

    KT = int(np.ceil(max(bin_cnt.max(), 1) / 128))
    # fake self-edges for empty slots keep their softmax denominator > 0
    n_fake = int((CAP - bin_n).sum())
    KT = int(np.ceil((bin_cnt + (CAP - bin_n)).max() / 128))
    NT = NB * KT

    core_of = bin_of // NB
    block_of = bin_of % NB
    loc_of = block_of * 128 + slot_of            # 0..SHP-1 within core
    g_of = core_of * SHP + loc_of                # global remapped id

    # edge -> (core, bin, seq within bin)
    e_bin = bin_of[dst_all]
    # fake edges: (bin, slot) pairs for empty slots
    fb, fs = [], []
    for b in range(NBINS):
        for s in range(bin_n[b], CAP):
            fb.append(b)
            fs.append(s)
    fb = np.array(fb, dtype=np.int64)
    fs = np.array(fs, dtype=np.int64)

    # per-edge values
    e_src_g = g_of[src_all]
    e_dst_slot = slot_of[dst_all].astype(np.int64)
    e_dst_loc = loc_of[dst_all].astype(np.int64)
    if len(fb):
        f_core = fb // NB
        f_block = fb % NB
        f_loc = f_block * 128 + fs
        f_g = f_core * SHP + f_loc
        e_bin = np.concatenate([e_bin, fb])
        e_src_g = np.concatenate([e_src_g, f_g])
        e_dst_slot = np.concatenate([e_dst_slot, fs])
        e_dst_loc = np.concatenate([e_dst_loc, f_loc])

    eo = np.argsort(e_bin, kind='stable')
    e_bin = e_bin[eo]
    e_src_g = e_src_g[eo]
    e_dst_slot = e_dst_slot[eo]
    e_dst_loc = e_dst_loc[eo]
    # sequence index within each bin
    starts = np.searchsorted(e_bin, np.arange(NBINS))
    seq = np.arange(len(e_bin)) - starts[e_bin]
    assert seq.max() < KT * 128

    e_core = e_bin // NB
    e_block = e_bin % NB
    e_t = seq // 128
    e_lane = seq % 128
    e_j = e_block * KT + e_t

    src_idx = np.zeros((NCORES, 128, NT), dtype=np.int32)
    dl_idx = np.zeros((NCORES, 128, NT), dtype=np.int32)
    masks = np.zeros((NCORES, 128, NT, 128), dtype=bfloat16)
    masksT = np.zeros((NCORES, 128, NT, 128), dtype=bfloat16)
    src_idx[e_core, e_lane, e_j] = _chunk_remap(
        e_src_g // SHP, e_src_g % SHP)
    dl_idx[e_core, e_lane, e_j] = e_dst_loc  # local within core
    masks[e_core, e_lane, e_j, e_dst_slot] = 1.0   # [lane -> dst slot]
    masksT[e_core, e_dst_slot, e_j, e_lane] = 1.0  # [dst slot -> lane]
    masks = masks.reshape(NCORES, 128, NT * 128)
    masksT = masksT.reshape(NCORES, 128, NT * 128)

    # permuted node data
    degc = np.clip(np.asarray(deg, dtype=np.int64), 0, MAXDEG)
    degg = (deg_emb[degc] + b_in[None, :]).astype(np.float32)
    xT = np.zeros((NCORES, IN_C, SHP), dtype=bfloat16)
    dege = np.zeros((NCORES, SHP, HID), dtype=np.float32)
    dege[:, :, :] = b_in[None, None, :]
    pool_oh = np.zeros((NCORES, NB, 128, G), dtype=bfloat16)

    xf = np.asarray(x, dtype=np.float32)
    bt = np.asarray(batch, dtype=np.int64)
    cores = core_of.astype(np.int64)
    xT[cores, :, loc_of] = xf.astype(bfloat16)
    dege[cores, loc_of, :] = degg
    cnt = np.bincount(bt, minlength=G).astype(np.float32)
    inv1d = (1.0 / np.maximum(cnt, 1.0)).astype(np.float32)
    # mean-pool weights folded into the one-hot
    pool_oh[cores, block_of, slot_of, bt] = inv1d[bt].astype(bfloat16)
    inv_cnt = inv1d.reshape(G, 1)
    return (KT, src_idx, dl_idx, masks, masksT, xT, dege.astype(bfloat16),
            pool_oh, inv_cnt)


def _emulate(prep, Win, Wl, Wr, att, b_conv, gamma, beta, Wc, b_c):
    """Numpy emulation of the device program (fp32) to validate host prep."""
    KT, src_idx, dl_idx, masks, masksT, xT, dege, pool_oh, inv_cnt = prep
    NT = NB * KT
    masks = masks.astype(np.float32).reshape(NCORES, 128, NT, 128)
    h = np.zeros((NCORES, SHP, HID), dtype=np.float32)
    for c in range(NCORES):
        h[c] = xT[c].astype(np.float32).T @ Win + dege[c].astype(np.float32)
    for l in range(L):
        xl = np.einsum('csh,hf->csf', h, Wl[l])
        xr = np.einsum('csh,hf->csf', h, Wr[l])
        # chunk-major AllGather layout
        xl_full = np.concatenate(
            [xl[:, CHB[c] * 128:CHB[c + 1] * 128, :].reshape(-1, HID)
             for c in range(4)], axis=0)
        h_new = np.zeros_like(h)
        for c in range(NCORES):
            for b in range(NB):
                acc = np.zeros((128, HID + HEADS), dtype=np.float32)
                for t in range(KT):
                    j = b * KT + t
                    z = xl_full[src_idx[c, :, j]] + xr[c][dl_idx[c, :, j]]
                    zl = np.maximum(z, NEG * z)
                    wat = zl * np.repeat(att[l].reshape(1, HID), 128, 0)
                    logit = wat.reshape(128, HEADS, HD).sum(-1)
                    ew = np.exp(logit)
                    wm = np.concatenate(
                        [(z.reshape(128, HEADS, HD) * ew[:, :, None]).reshape(128, HID), ew], 1)
                    acc += masks[c, :, j, :].T @ wm
                rcp = 1.0 / acc[:, HID:]
                v = (acc[:, :HID].reshape(128, HEADS, HD) * rcp[:, :, None]).reshape(128, HID)
                v = v - xr[c][b * 128:(b + 1) * 128] + b_conv[l][None, :]
                v = np.maximum(v, 0.0) + h[c][b * 128:(b + 1) * 128]
                mu = v.mean(-1, keepdims=True)
                var = v.var(-1, keepdims=True)
                hn = (v - mu) / np.sqrt(var + 1e-5) * gamma[l][None, :] + beta[l][None, :]
                h_new[c][b * 128:(b + 1) * 128] = hn
        h = h_new
    pooled = np.zeros((G, HID), dtype=np.float32)
    for c in range(NCORES):
        pooled += pool_oh[c].astype(np.float32).reshape(SHP, G).T @ h[c]
    return pooled @ Wc + b_c[None, :]


def kernel(**inputs):
    from concourse import bass_utils
    from ml_dtypes import bfloat16
    x = np.asarray(inputs["x"], dtype=np.float32)
    edge_index = np.asarray(inputs["edge_index"])
    batch = np.asarray(inputs["batch"])
    deg = np.asarray(inputs["deg"])
    Win = np.asarray(inputs["Win"], dtype=np.float32)
    b_in = np.asarray(inputs["b_in"], dtype=np.float32)
    deg_emb = np.asarray(inputs["deg_emb"], dtype=np.float32)
    Wl = np.asarray(inputs["Wl"], dtype=np.float32)
    Wr = np.asarray(inputs["Wr"], dtype=np.float32)
    att = np.asarray(inputs["att"], dtype=np.float32)
    b_conv = np.asarray(inputs["b_conv"], dtype=np.float32)
    gamma = np.asarray(inputs["gamma"], dtype=np.float32)
    beta = np.asarray(inputs["beta"], dtype=np.float32)
    Wc = np.asarray(inputs["Wc"], dtype=np.float32)
    b_c = np.asarray(inputs["b_c"], dtype=np.float32)

    prep = _host_prep(edge_index, batch, deg, deg_emb, b_in, x)
    KT, src_idx, dl_idx, masks, masksT, xT, dege, pool_oh, inv_cnt = prep

    if KT not in _CACHE:
        _CACHE[KT] = _build(KT)
    nc = _CACHE[KT]

    att_rep = np.tile(att.reshape(L, 1, HID), (1, 128, 2)).astype(bfloat16)
    gam_rep = np.repeat(gamma.reshape(L, 1, HID), 128, axis=1).astype(bfloat16)
    bet_rep = np.repeat(beta.reshape(L, 1, HID), 128, axis=1).astype(bfloat16)
    bcv_rep = np.repeat(b_conv.reshape(L, 1, HID), 128, axis=1).astype(bfloat16)
    bc_rep = np.repeat(b_c.reshape(1, NCLS), G, axis=0).astype(np.float32)

    in_maps = []
    for c in range(NCORES):
        in_maps.append({
            "xT": xT[c], "dege": dege[c], "sidx": src_idx[c],
            "msk": masks[c], "mskT": masksT[c],
            "Win": Win.astype(bfloat16), "Wl": Wl.astype(bfloat16),
            "Wr": Wr.astype(bfloat16), "attr": att_rep, "gamr": gam_rep,
            "betr": bet_rep, "bcvr": bcv_rep,
            "poh": pool_oh[c], "icnt": inv_cnt, "Wc": Wc, "bcr": bc_rep,
        })
    res = bass_utils.run_bass_kernel_spmd(nc, in_maps, core_ids=list(range(NCORES)))
    kernel.last_result = res
    return res.results[0]["out"].astype(np.float32)


def _build(KT):
    from concourse import bass, mybir, tile, bacc
    from concourse.masks import make_identity
    F32 = mybir.dt.float32
    BF16 = mybir.dt.bfloat16
    I32 = mybir.dt.int32
    AF = mybir.ActivationFunctionType
    OP = mybir.AluOpType
    NT = NB * KT

    nc = bacc.Bacc("TRN2", target_bir_lowering=False, debug=False,
                   enable_asserts=False, num_devices=NCORES)

    t_xT = nc.dram_tensor("xT", [IN_C, SHP], BF16, kind="ExternalInput").ap()
    t_dege = nc.dram_tensor("dege", [SHP, HID], BF16, kind="ExternalInput").ap()
    t_sidx = nc.dram_tensor("sidx", [128, NT], I32, kind="ExternalInput").ap()
    t_msk = nc.dram_tensor("msk", [128, NT * 128], BF16, kind="ExternalInput").ap()
    t_mskT = nc.dram_tensor("mskT", [128, NT * 128], BF16, kind="ExternalInput").ap()
    t_Win = nc.dram_tensor("Win", [IN_C, HID], BF16, kind="ExternalInput").ap()
    t_Wl = nc.dram_tensor("Wl", [L, HID, HID], BF16, kind="ExternalInput").ap()
    t_Wr = nc.dram_tensor("Wr", [L, HID, HID], BF16, kind="ExternalInput").ap()
    t_att = nc.dram_tensor("attr", [L, 128, 2 * HID], BF16, kind="ExternalInput").ap()
    t_gam = nc.dram_tensor("gamr", [L, 128, HID], BF16, kind="ExternalInput").ap()
    t_bet = nc.dram_tensor("betr", [L, 128, HID], BF16, kind="ExternalInput").ap()
    t_bcv = nc.dram_tensor("bcvr", [L, 128, HID], BF16, kind="ExternalInput").ap()
    t_poh = nc.dram_tensor("poh", [NB, 128, G], BF16, kind="ExternalInput").ap()
    t_icnt = nc.dram_tensor("icnt", [G, 1], F32, kind="ExternalInput").ap()
    t_Wc = nc.dram_tensor("Wc", [HID, NCLS], F32, kind="ExternalInput").ap()
    t_bc = nc.dram_tensor("bcr", [G, NCLS], F32, kind="ExternalInput").ap()
    t_out = nc.dram_tensor("out", [G, NCLS], F32, kind="ExternalOutput").ap()

    with tile.TileContext(nc) as tc:
        from contextlib import ExitStack
        with ExitStack() as ctx:
            cpool = ctx.enter_context(tc.tile_pool(name="const", bufs=1))
            dram = ctx.enter_context(tc.tile_pool(name="dram", bufs=1, space="DRAM"))
            sb = ctx.enter_context(tc.tile_pool(name="sb", bufs=6))
            gp = ctx.enter_context(tc.tile_pool(name="gp", bufs=6))
            mp = ctx.enter_context(tc.tile_pool(name="mp", bufs=2))
            sres = ctx.enter_context(tc.tile_pool(name="sres", bufs=1))
            ps = ctx.enter_context(tc.tile_pool(name="ps", bufs=3, space="PSUM"))
            pt = ctx.enter_context(tc.tile_pool(name="pt", bufs=1, space="PSUM"))
            pacc = ctx.enter_context(tc.tile_pool(name="pacc", bufs=2, space="PSUM"))
            ppool = ctx.enter_context(tc.tile_pool(name="ppool", bufs=1, space="PSUM"))
            pz = ps

            identb = cpool.tile([128, 128], BF16)
            make_identity(nc, identb[:])
            epsc = cpool.tile([128, 1], F32)
            nc.vector.memset(epsc[:], 1e-5)

            sidx = sres.tile([128, NT], I32)
            nc.sync.dma_start(sidx[:], t_sidx)

            # h resident double buffer + plain xr resident
            h_res = [sres.tile([128, NB * HID], BF16, name=f"h_res{i}")
                     for i in range(2)]
            xr_res = sres.tile([128, NB * HID], BF16)

            xl_sh = dram.tile([SHP, HID], BF16)
            xl_full = dram.tile([NTOT, HID], BF16)

            # ---- stage A: h0 = x @ Win + (deg_emb[deg] + b_in) ----
            winsb = cpool.tile([IN_C, HID], BF16)
            nc.sync.dma_start(winsb[:], t_Win)
            for nt in range(NB):
                xtt = sb.tile([IN_C, 128], BF16, tag="xtt")
                nc.sync.dma_start(xtt[:], t_xT[:, nt * 128:(nt + 1) * 128])
                p0 = ps.tile([128, 2 * HID], F32, tag="mm", space="PSUM")
                nc.tensor.matmul(out=p0[:, :HID], lhsT=xtt[:], rhs=winsb[:],
                                 start=True, stop=True)
                dg = sb.tile([128, HID], BF16, tag="dg")
                nc.sync.dma_start(dg[:], t_dege[nt * 128:(nt + 1) * 128, :])
                nc.vector.tensor_tensor(out=h_res[0][:, nt * HID:(nt + 1) * HID],
                                        in0=p0[:, :HID], in1=dg[:], op=OP.add)

            for l in range(L):
                hc = h_res[l % 2]
                hn_res = h_res[(l + 1) % 2]
                attr = cpool.tile([128, 2 * HID], BF16, tag=f"attr{l}")
                nc.sync.dma_start(attr[:], t_att[l])
                gamr = cpool.tile([128, HID], BF16, tag=f"gamr{l}")
                nc.sync.dma_start(gamr[:], t_gam[l])
                betr = cpool.tile([128, HID], BF16, tag=f"betr{l}")
                nc.sync.dma_start(betr[:], t_bet[l])
                bcvr = cpool.tile([128, HID], BF16, tag=f"bcvr{l}")
                nc.sync.dma_start(bcvr[:], t_bcv[l])
                wlr = cpool.tile([128, 2, 2 * HID], BF16, tag=f"wlr{l}")
                for half in range(2):
                    nc.sync.dma_start(wlr[:, half, :HID],
                                      t_Wl[l, half * 128:(half + 1) * 128, :])
                    nc.sync.dma_start(wlr[:, half, HID:],
                                      t_Wr[l, half * 128:(half + 1) * 128, :])

                # ---- B1: xl/xr for own shard; AllGather per chunk as soon
                # as its blocks are done so the collective overlaps B1 ----
                off = 0
                for nt in range(NB):
                    hT = sb.tile([128, HID], BF16, tag="hT")
                    for half in range(2):
                        tp = pt.tile([128, 128], BF16, tag="t128", space="PSUM")
                        nc.tensor.transpose(
                            out=tp[:],
                            in_=hc[:, nt * HID + half * 128:nt * HID + (half + 1) * 128],
                            identity=identb[:])
                        nc.scalar.activation(out=hT[:, half * 128:(half + 1) * 128],
                                             in_=tp[:], func=AF.Copy)
                    pxx = ps.tile([128, 2 * HID], F32, tag="mm", space="PSUM")
                    for half in range(2):
                        nc.tensor.matmul(out=pxx[:],
                                         lhsT=hT[:, half * 128:(half + 1) * 128],
                                         rhs=wlr[:, half, :],
                                         start=half == 0, stop=half == 1)
                    xl_o = sb.tile([128, HID], BF16, tag="xl_o")
                    nc.scalar.activation(out=xl_o[:], in_=pxx[:, :HID], func=AF.Copy)
                    nc.sync.dma_start(xl_sh[nt * 128:(nt + 1) * 128, :], xl_o[:])
                    nc.scalar.activation(out=xr_res[:, nt * HID:(nt + 1) * HID],
                                         in_=pxx[:, HID:], func=AF.Copy)
                    if nt + 1 in CHB:
                        ci = CHB.index(nt + 1) - 1
                        r0, r1 = CHB[ci] * 128, CHB[ci + 1] * 128
                        nrows = (r1 - r0) * NCORES
                        nc.gpsimd.collective_compute(
                            "AllGather", OP.bypass,
                            replica_groups=[list(range(NCORES))],
                            ins=[xl_sh[r0:r1, :].opt()],
                            outs=[xl_full[off:off + nrows, :].opt()])
                        off += nrows

                # ---- B3: edge phase (wat/reduce fused across tile pairs) ----
                mb = None
                mbT = None
                for b in range(NB):
                  for tp in range(0, KT, 2):
                    npair = min(2, KT - tp)
                    W = npair * HID
                    zts = []
                    for k in range(npair):
                        t = tp + k
                        j = b * KT + t
                        if j % GBM == 0:
                            mn = min(GBM, NT - j)
                            mbT = mp.tile([128, GBM * 128], BF16, tag="mbT")
                            nc.sync.dma_start(mbT[:, :mn * 128],
                                              t_mskT[:, j * 128:(j + mn) * 128])
                        xe = gp.tile([128, HID], BF16, tag="xe")
                        nc.gpsimd.indirect_dma_start(
                            out=xe[:], out_offset=None, in_=xl_full[:],
                            in_offset=bass.IndirectOffsetOnAxis(
                                ap=sidx[:, j:j + 1], axis=0))
                        # z = xr[dl] + xl[src] assembled in PSUM
                        zt = pz.tile([128, HID], F32, tag="mm", space="PSUM",
                                     name=f"zt{k}")
                        nc.tensor.matmul(out=zt[:],
                                         lhsT=mbT[:, (j % GBM) * 128:(j % GBM + 1) * 128],
                                         rhs=xr_res[:, b * HID:(b + 1) * HID],
                                         start=True, stop=False)
                        nc.tensor.matmul(out=zt[:], lhsT=identb[:], rhs=xe[:],
                                         start=False, stop=True)
                        zts.append(zt)
                    zl2 = sb.tile([128, 2 * HID], BF16, tag="zl2")
                    for k in range(npair):
                        nc.scalar.activation(out=zl2[:, k * HID:(k + 1) * HID],
                                             in_=zts[k][:], func=AF.Prelu,
                                             alpha=NEG)
                    wat2 = sb.tile([128, 2 * HID], BF16, tag="wat2")
                    nc.vector.tensor_tensor(out=wat2[:, :W], in0=zl2[:, :W],
                                            in1=attr[:, :W], op=OP.mult)
                    lg2 = sb.tile([128, 2 * HEADS], F32, tag="lg2")
                    nc.vector.tensor_reduce(
                        out=lg2[:, :npair * HEADS],
                        in_=wat2[:, :W].rearrange("p (m d) -> p m d", d=HD),
                        axis=mybir.AxisListType.X, op=OP.add)
                    for k in range(npair):
                        t = tp + k
                        j = b * KT + t
                        if j % GBM == 0:
                            mn = min(GBM, NT - j)
                            mb = mp.tile([128, GBM * 128], BF16, tag="mb")
                            nc.sync.dma_start(mb[:, :mn * 128],
                                              t_msk[:, j * 128:(j + mn) * 128])
                        wm = sb.tile([128, HID + HEADS], BF16, tag="wm")
                        nc.scalar.activation(
                            out=wm[:, HID:],
                            in_=lg2[:, k * HEADS:(k + 1) * HEADS], func=AF.Exp)
                        nc.vector.tensor_tensor(
                            out=wm[:, :HID].rearrange("p (h d) -> p h d", h=HEADS),
                            in0=zts[k][:].rearrange("p (h d) -> p h d", h=HEADS),
                            in1=wm[:, HID:].broadcast_to([128, HEADS, HD]),
                            op=OP.mult)
                        if t == 0:
                            acc = pacc.tile([128, HID + HEADS], F32, tag="acc",
                                            space="PSUM")
                            _build.acc = acc
                        nc.tensor.matmul(out=_build.acc[:],
                                         lhsT=mb[:, (j % GBM) * 128:(j % GBM + 1) * 128],
                                         rhs=wm[:], start=t == 0, stop=t == KT - 1)
                  if True:
                    # ---- finalize block b ----
                    acc = _build.acc
                    rcp = sb.tile([128, HEADS], F32, tag="rcp")
                    nc.vector.reciprocal(rcp[:], acc[:, HID:])
                    v = sb.tile([128, HID], F32, tag="v")
                    nc.vector.tensor_tensor(
                        out=v[:].rearrange("p (h d) -> p h d", h=HEADS),
                        in0=acc[:, :HID].rearrange("p (h d) -> p h d", h=HEADS),
                        in1=rcp[:].broadcast_to([128, HEADS, HD]), op=OP.mult)
                    vx = sb.tile([128, HID], F32, tag="vx")
                    nc.vector.tensor_tensor(out=vx[:], in0=v[:],
                                            in1=xr_res[:, b * HID:(b + 1) * HID],
                                            op=OP.subtract)
                    vb = sb.tile([128, HID], F32, tag="vb")
                    nc.vector.tensor_tensor(out=vb[:], in0=vx[:], in1=bcvr[:],
                                            op=OP.add)
                    vr = sb.tile([128, HID], F32, tag="vr")
                    nc.vector.scalar_tensor_tensor(
                        out=vr[:], in0=vb[:], scalar=0.0,
                        in1=hc[:, b * HID:(b + 1) * HID],
                        op0=OP.max, op1=OP.add)
                    st6 = sb.tile([128, 6], F32, tag="st6")
                    nc.vector.bn_stats(st6[:], vr[:])
                    mv = sb.tile([128, 2], F32, tag="mv")
                    nc.vector.bn_aggr(mv[:], st6[:])
                    lnv = sb.tile([128, 1], F32, tag="lnv")
                    nc.scalar.activation(out=lnv[:], in_=mv[:, 1:2], func=AF.Ln,
                                         bias=epsc[:, :1])
                    rstd = sb.tile([128, 1], F32, tag="rstd")
                    nc.scalar.activation(out=rstd[:], in_=lnv[:], func=AF.Exp,
                                         scale=-0.5)
                    hng = sb.tile([128, HID], F32, tag="hng")
                    nc.vector.scalar_tensor_tensor(
                        out=hng[:], in0=vr[:], scalar=mv[:, 0:1], in1=gamr[:],
                        op0=OP.subtract, op1=OP.mult)
                    nc.vector.scalar_tensor_tensor(
                        out=hn_res[:, b * HID:(b + 1) * HID], in0=hng[:],
                        scalar=rstd[:, :1], in1=betr[:],
                        op0=OP.mult, op1=OP.add)
                    if l == L - 1:
                        # pool transposed: poolT[f, g] += hn[n, f]^T @ poh[n, g]
                        poh = sb.tile([128, G], BF16, tag="poh")
                        nc.sync.dma_start(poh[:], t_poh[b])
                        if b == 0:
                            ppA = ppool.tile([128, G], F32, tag="ppA",
                                             space="PSUM")
                            ppB = ppool.tile([128, G], F32, tag="ppB",
                                             space="PSUM")
                            _build.pp = (ppA, ppB)
                        for half in range(2):
                            nc.tensor.matmul(
                                out=_build.pp[half][:],
                                lhsT=hn_res[:, b * HID + half * 128:
                                            b * HID + (half + 1) * 128],
                                rhs=poh[:], start=b == 0, stop=b == NB - 1)

            # ---- stage C: pool (transposed layout) + classifier ----
            pool_sb = sb.tile([128, 2 * G], F32, tag="pool_sb")
            nc.scalar.activation(out=pool_sb[:, :G], in_=_build.pp[0][:], func=AF.Copy)
            nc.scalar.activation(out=pool_sb[:, G:], in_=_build.pp[1][:], func=AF.Copy)
            pl_in = dram.tile([128, 2 * G], F32)
            pl_out = dram.tile([128, 2 * G], F32)
            nc.sync.dma_start(pl_in[:], pool_sb[:])
            nc.gpsimd.collective_compute(
                "AllReduce", OP.add, replica_groups=[list(range(NCORES))],
                ins=[pl_in[:].opt()], outs=[pl_out[:].opt()])
            poolc = sb.tile([128, 2 * G], F32, tag="poolc")
            nc.sync.dma_start(poolc[:], pl_out[:])
            wc_sb = sb.tile([128, 2 * NCLS], F32, tag="wc")
            for half in range(2):
                nc.sync.dma_start(wc_sb[:, half * NCLS:(half + 1) * NCLS],
                                  t_Wc[half * 128:(half + 1) * 128, :])
            pcls = ppool.tile([G, NCLS], F32, tag="ppA", space="PSUM")
            for half in range(2):
                nc.tensor.matmul(out=pcls[:],
                                 lhsT=poolc[:, half * G:(half + 1) * G],
                                 rhs=wc_sb[:, half * NCLS:(half + 1) * NCLS],
                                 start=half == 0, stop=half == 1)
            bc_sb = sb.tile([G, NCLS], F32, tag="bc")
            nc.sync.dma_start(bc_sb[:], t_bc)
            res = sb.tile([G, NCLS], F32, tag="resout")
            nc.vector.tensor_tensor(out=res[:], in0=pcls[:], in1=bc_sb[:], op=OP.add)
            nc.sync.dma_start(t_out, res[:])

    nc.compile()
    return nc


# revision 51
# speedup vs baseline: 1.0862x; 1.0862x over previous
"""GATv2 GNN (Graphormer-style) on 8 trn2 NeuronCores.

Strategy: nodes are re-numbered via LPT bin-packing so every 128-node dst
block has <= KT*128 incoming edges (KT=17, near-zero padding) and every core
gets an equal edge count. Per layer: each core computes xl/xr for its shard
(bf16 matmuls), AllGathers xl (bf16), then streams edge tiles: batched
indirect-DMA gathers fetch z = xl[src] (+ xr[dst] fused via compute_op=add),
leaky-relu on GpSimd, per-head logits via one Vector 3D-reduce, Exp on
Scalar, exp-weighted messages scattered into PSUM with one bf16 matmul
against a host-precomputed one-hot mask. Softmax max-subtraction is skipped
(logits are O(10); exact in fp32 since a per-dst constant cancels).
Aggregation uses sum(alpha*z) - xr[d] (valid since sum(alpha)=1), so xl
never needs a second gather. h stays SBUF-resident across layers.
"""
import sys
import numpy as np

sys.path.insert(0, '/opt/trn_rl_repo')

N, E, IN_C, HID, HEADS, L, G, NCLS, MAXDEG = 50000, 800000, 128, 256, 4, 2, 64, 10, 10
HD = HID // HEADS
NCORES = 8
NB = 49                     # dst blocks per core
SHP = NB * 128              # 6272 node slots per core
NTOT = NCORES * SHP
NEG = 0.2
GBZ = 32                    # tiles per batched gather
GBM = 32                    # tiles per batched mask load
CHB = [0, 13, 25, 37, 49]   # block boundaries of AllGather chunks

_CACHE = {}


def _chunk_remap(core, loc):
    """Map (core, local row) -> row in chunk-major xl_full2 layout."""
    block = loc // 128
    chunk = np.digitize(block, CHB[1:4])
    start = np.array(CHB, dtype=np.int64)[chunk] * 128
    rows = (np.array(CHB[1:], dtype=np.int64) -
            np.array(CHB[:-1], dtype=np.int64))[chunk] * 128
    off = np.array([0] + list(np.cumsum(
        [(CHB[i + 1] - CHB[i]) * 128 * NCORES for i in range(4)])[:3]),
        dtype=np.int64)[chunk]
    return off + core * rows + (loc - start)


def _host_prep(edge_index, batch, deg, deg_emb, b_in, x):
    import heapq
    from ml_dtypes import bfloat16

    src0 = np.asarray(edge_index[0], dtype=np.int64)
    dst0 = np.asarray(edge_index[1], dtype=np.int64)
    loops = np.arange(N, dtype=np.int64)
    src_all = np.concatenate([src0, loops])
    dst_all = np.concatenate([dst0, loops])

    indeg = np.bincount(dst_all, minlength=N)
    NBINS = NCORES * NB
    CAP = 128

    # LPT: heaviest nodes first into the lightest (by edge count) non-full bin
    order = np.argsort(-indeg, kind='stable')
    bin_cnt = np.zeros(NBINS, dtype=np.int64)
    bin_n = np.zeros(NBINS, dtype=np.int64)
    bin_of = np.zeros(N, dtype=np.int32)
    slot_of = np.zeros(N, dtype=np.int32)
    heap = [(0, b) for b in range(NBINS)]
    heapq.heapify(heap)
    for i in order:
        while True:
            c, b = heapq.heappop(heap)
            if c != bin_cnt[b] or bin_n[b] >= CAP:
                continue
            break
        bin_of[i] = b
        slot_of[i] = bin_n[b]
        bin_n[b] += 1
        bin_cnt[b] += indeg[i]
        if bin_n[b] < CAP:
            heapq.heappush(heap, (int(bin_cnt[b]), b))

    KT = int(np.ceil(max(bin_cnt.max(), 1) / 128))
    # fake self-edges for empty slots keep their softmax denominator > 0
    n_fake = int((CAP - bin_n).sum())
    KT = int(np.ceil((bin_cnt + (CAP - bin_n)).max() / 128))
    NT = NB * KT

    core_of = bin_of // NB
    block_of = bin_of % NB
    loc_of = block_of * 128 + slot_of            # 0..SHP-1 within core
    g_of = core_of * SHP + loc_of                # global remapped id

    # edge -> (core, bin, seq within bin)
    e_bin = bin_of[dst_all]
    # fake edges: (bin, slot) pairs for empty slots
    fb, fs = [], []
    for b in range(NBINS):
        for s in range(bin_n[b], CAP):
            fb.append(b)
            fs.append(s)
    fb = np.array(fb, dtype=np.int64)
    fs = np.array(fs, dtype=np.int64)

    # per-edge values
    e_src_g = g_of[src_all]
    e_dst_slot = slot_of[dst_all].astype(np.int64)
    e_dst_loc = loc_of[dst_all].astype(np.int64)
    if len(fb):
        f_core = fb // NB
        f_block = fb % NB
        f_loc = f_block * 128 + fs
        f_g = f_core * SHP + f_loc
        e_bin = np.concatenate([e_bin, fb])
        e_src_g = np.concatenate([e_src_g, f_g])
        e_dst_slot = np.concatenate([e_dst_slot, fs])
        e_dst_loc = np.concatenate([e_dst_loc, f_loc])

    eo = np.argsort(e_bin, kind='stable')
    e_bin = e_bin[eo]
    e_src_g = e_src_g[eo]
    e_dst_slot = e_dst_slot[eo]
    e_dst_loc = e_dst_loc[eo]
    # sequence index within each bin
    starts = np.searchsorted(e_bin, np.arange(NBINS))
    seq = np.arange(len(e_bin)) - starts[e_bin]
    assert seq.max() < KT * 128

    e_core = e_bin // NB
    e_block = e_bin % NB
    e_t = seq // 128
    e_lane = seq % 128
    e_j = e_block * KT + e_t

    src_idx = np.zeros((NCORES, 128, NT), dtype=np.int32)
    dl_idx = np.zeros((NCORES, 128, NT), dtype=np.int32)
    masks = np.zeros((NCORES, 128, NT, 128), dtype=bfloat16)
    masksT = np.zeros((NCORES, 128, NT, 128), dtype=bfloat16)
    src_idx[e_core, e_lane, e_j] = _chunk_remap(
        e_src_g // SHP, e_src_g % SHP)
    dl_idx[e_core, e_lane, e_j] = e_dst_loc  # local within core
    masks[e_core, e_lane, e_j, e_dst_slot] = 1.0   # [lane -> dst slot]
    masksT[e_core, e_dst_slot, e_j, e_lane] = 1.0  # [dst slot -> lane]
    masks = masks.reshape(NCORES, 128, NT * 128)
    masksT = masksT.reshape(NCORES, 128, NT * 128)

    # permuted node data
    degc = np.clip(np.asarray(deg, dtype=np.int64), 0, MAXDEG)
    degg = (deg_emb[degc] + b_in[None, :]).astype(np.float32)
    xT = np.zeros((NCORES, IN_C, SHP), dtype=bfloat16)
    dege = np.zeros((NCORES, SHP, HID), dtype=np.float32)
    dege[:, :, :] = b_in[None, None, :]
    pool_oh = np.zeros((NCORES, NB, 128, G), dtype=bfloat16)

    xf = np.asarray(x, dtype=np.float32)
    bt = np.asarray(batch, dtype=np.int64)
    cores = core_of.astype(np.int64)
    xT[cores, :, loc_of] = xf.astype(bfloat16)
    dege[cores, loc_of, :] = degg
    cnt = np.bincount(bt, minlength=G).astype(np.float32)
    inv1d = (1.0 / np.maximum(cnt, 1.0)).astype(np.float32)
    # mean-pool weights folded into the one-hot
    pool_oh[cores, block_of, slot_of, bt] = inv1d[bt].astype(bfloat16)
    inv_cnt = inv1d.reshape(G, 1)
    return (KT, src_idx, dl_idx, masks, masksT, xT, dege.astype(bfloat16),
            pool_oh, inv_cnt)


def _emulate(prep, Win, Wl, Wr, att, b_conv, gamma, beta, Wc, b_c):
    """Numpy emulation of the device program (fp32) to validate host prep."""
    KT, src_idx, dl_idx, masks, masksT, xT, dege, pool_oh, inv_cnt = prep
    NT = NB * KT
    masks = masks.astype(np.float32).reshape(NCORES, 128, NT, 128)
    h = np.zeros((NCORES, SHP, HID), dtype=np.float32)
    for c in range(NCORES):
        h[c] = xT[c].astype(np.float32).T @ Win + dege[c].astype(np.float32)
    for l in range(L):
        xl = np.einsum('csh,hf->csf', h, Wl[l])
        xr = np.einsum('csh,hf->csf', h, Wr[l])
        # chunk-major AllGather layout
        xl_full = np.concatenate(
            [xl[:, CHB[c] * 128:CHB[c + 1] * 128, :].reshape(-1, HID)
             for c in range(4)], axis=0)
        h_new = np.zeros_like(h)
        for c in range(NCORES):
            for b in range(NB):
                acc = np.zeros((128, HID + HEADS), dtype=np.float32)
                for t in range(KT):
                    j = b * KT + t
                    z = xl_full[src_idx[c, :, j]] + xr[c][dl_idx[c, :, j]]
                    zl = np.maximum(z, NEG * z)
                    wat = zl * np.repeat(att[l].reshape(1, HID), 128, 0)
                    logit = wat.reshape(128, HEADS, HD).sum(-1)
                    ew = np.exp(logit)
                    wm = np.concatenate(
                        [(z.reshape(128, HEADS, HD) * ew[:, :, None]).reshape(128, HID), ew], 1)
                    acc += masks[c, :, j, :].T @ wm
                rcp = 1.0 / acc[:, HID:]
                v = (acc[:, :HID].reshape(128, HEADS, HD) * rcp[:, :, None]).reshape(128, HID)
                v = v - xr[c][b * 128:(b + 1) * 128] + b_conv[l][None, :]
                v = np.maximum(v, 0.0) + h[c][b * 128:(b + 1) * 128]
                mu = v.mean(-1, keepdims=True)
                var = v.var(-1, keepdims=True)
                hn = (v - mu) / np.sqrt(var + 1e-5) * gamma[l][None, :] + beta[l][None, :]
                h_new[c][b * 128:(b + 1) * 128] = hn
        h = h_new
    pooled = np.zeros((G, HID), dtype=np.float32)
    for c in range(NCORES):
        pooled += pool_oh[c].astype(np.float32).reshape(SHP, G).T @ h[c]
    return pooled @ Wc + b_c[None, :]


def kernel(**inputs):
    from concourse import bass_utils
    from ml_dtypes import bfloat16
    x = np.asarray(inputs["x"], dtype=np.float32)
    edge_index = np.asarray(inputs["edge_index"])
    batch = np.asarray(inputs["batch"])
    deg = np.asarray(inputs["deg"])
    Win = np.asarray(inputs["Win"], dtype=np.float32)
    b_in = np.asarray(inputs["b_in"], dtype=np.float32)
    deg_emb = np.asarray(inputs["deg_emb"], dtype=np.float32)
    Wl = np.asarray(inputs["Wl"], dtype=np.float32)
    Wr = np.asarray(inputs["Wr"], dtype=np.float32)
    att = np.asarray(inputs["att"], dtype=np.float32)
    b_conv = np.asarray(inputs["b_conv"], dtype=np.float32)
    gamma = np.asarray(inputs["gamma"], dtype=np.float32)
    beta = np.asarray(inputs["beta"], dtype=np.float32)
    Wc = np.asarray(inputs["Wc"], dtype=np.float32)
    b_c = np.asarray(inputs["b_c"], dtype=np.float32)

    prep = _host_prep(edge_index, batch, deg, deg_emb, b_in, x)
    KT, src_idx, dl_idx, masks, masksT, xT, dege, pool_oh, inv_cnt = prep

    if KT not in _CACHE:
        _CACHE[KT] = _build(KT)
    nc = _CACHE[KT]

    att_rep = np.repeat(att.reshape(L, 1, HID), 128, axis=1).astype(bfloat16)
    gam_rep = np.repeat(gamma.reshape(L, 1, HID), 128, axis=1).astype(bfloat16)
    bet_rep = np.repeat(beta.reshape(L, 1, HID), 128, axis=1).astype(bfloat16)
    bcv_rep = np.repeat(b_conv.reshape(L, 1, HID), 128, axis=1).astype(bfloat16)
    bc_rep = np.repeat(b_c.reshape(1, NCLS), G, axis=0).astype(np.float32)

    in_maps = []
    for c in range(NCORES):
        in_maps.append({
            "xT": xT[c], "dege": dege[c], "sidx": src_idx[c],
            "msk": masks[c], "mskT": masksT[c],
            "Win": Win.astype(bfloat16), "Wl": Wl.astype(bfloat16),
            "Wr": Wr.astype(bfloat16), "attr": att_rep, "gamr": gam_rep,
            "betr": bet_rep, "bcvr": bcv_rep,
            "poh": pool_oh[c], "icnt": inv_cnt, "Wc": Wc, "bcr": bc_rep,
        })
    res = bass_utils.run_bass_kernel_spmd(nc, in_maps, core_ids=list(range(NCORES)))
    kernel.last_result = res
    return res.results[0]["out"].astype(np.float32)


def _build(KT):
    from concourse import bass, mybir, tile, bacc
    from concourse.masks import make_identity
    F32 = mybir.dt.float32
    BF16 = mybir.dt.bfloat16
    I32 = mybir.dt.int32
    AF = mybir.ActivationFunctionType
    OP = mybir.AluOpType
    NT = NB * KT

    nc = bacc.Bacc("TRN2", target_bir_lowering=False, debug=False,
                   enable_asserts=False, num_devices=NCORES)

    t_xT = nc.dram_tensor("xT", [IN_C, SHP], BF16, kind="ExternalInput").ap()
    t_dege = nc.dram_tensor("dege", [SHP, HID], BF16, kind="ExternalInput").ap()
    t_sidx = nc.dram_tensor("sidx", [128, NT], I32, kind="ExternalInput").ap()
    t_msk = nc.dram_tensor("msk", [128, NT * 128], BF16, kind="ExternalInput").ap()
    t_mskT = nc.dram_tensor("mskT", [128, NT * 128], BF16, kind="ExternalInput").ap()
    t_Win = nc.dram_tensor("Win", [IN_C, HID], BF16, kind="ExternalInput").ap()
    t_Wl = nc.dram_tensor("Wl", [L, HID, HID], BF16, kind="ExternalInput").ap()
    t_Wr = nc.dram_tensor("Wr", [L, HID, HID], BF16, kind="ExternalInput").ap()
    t_att = nc.dram_tensor("attr", [L, 128, HID], BF16, kind="ExternalInput").ap()
    t_gam = nc.dram_tensor("gamr", [L, 128, HID], BF16, kind="ExternalInput").ap()
    t_bet = nc.dram_tensor("betr", [L, 128, HID], BF16, kind="ExternalInput").ap()
    t_bcv = nc.dram_tensor("bcvr", [L, 128, HID], BF16, kind="ExternalInput").ap()
    t_poh = nc.dram_tensor("poh", [NB, 128, G], BF16, kind="ExternalInput").ap()
    t_icnt = nc.dram_tensor("icnt", [G, 1], F32, kind="ExternalInput").ap()
    t_Wc = nc.dram_tensor("Wc", [HID, NCLS], F32, kind="ExternalInput").ap()
    t_bc = nc.dram_tensor("bcr", [G, NCLS], F32, kind="ExternalInput").ap()
    t_out = nc.dram_tensor("out", [G, NCLS], F32, kind="ExternalOutput").ap()

    with tile.TileContext(nc) as tc:
        from contextlib import ExitStack
        with ExitStack() as ctx:
            cpool = ctx.enter_context(tc.tile_pool(name="const", bufs=1))
            dram = ctx.enter_context(tc.tile_pool(name="dram", bufs=1, space="DRAM"))
            sb = ctx.enter_context(tc.tile_pool(name="sb", bufs=6))
            gp = ctx.enter_context(tc.tile_pool(name="gp", bufs=6))
            mp = ctx.enter_context(tc.tile_pool(name="mp", bufs=2))
            sres = ctx.enter_context(tc.tile_pool(name="sres", bufs=1))
            ps = ctx.enter_context(tc.tile_pool(name="ps", bufs=3, space="PSUM"))
            pt = ctx.enter_context(tc.tile_pool(name="pt", bufs=1, space="PSUM"))
            pacc = ctx.enter_context(tc.tile_pool(name="pacc", bufs=2, space="PSUM"))
            ppool = ctx.enter_context(tc.tile_pool(name="ppool", bufs=1, space="PSUM"))
            pz = ps

            identb = cpool.tile([128, 128], BF16)
            make_identity(nc, identb[:])
            epsc = cpool.tile([128, 1], F32)
            nc.vector.memset(epsc[:], 1e-5)

            sidx = sres.tile([128, NT], I32)
            nc.sync.dma_start(sidx[:], t_sidx)

            # h resident double buffer + plain xr resident
            h_res = [sres.tile([128, NB * HID], BF16, name=f"h_res{i}")
                     for i in range(2)]
            xr_res = sres.tile([128, NB * HID], BF16)

            xl_sh = dram.tile([SHP, HID], BF16)
            xl_full = dram.tile([NTOT, HID], BF16)

            # ---- stage A: h0 = x @ Win + (deg_emb[deg] + b_in) ----
            winsb = cpool.tile([IN_C, HID], BF16)
            nc.sync.dma_start(winsb[:], t_Win)
            for nt in range(NB):
                xtt = sb.tile([IN_C, 128], BF16, tag="xtt")
                nc.sync.dma_start(xtt[:], t_xT[:, nt * 128:(nt + 1) * 128])
                p0 = ps.tile([128, 2 * HID], F32, tag="mm", space="PSUM")
                nc.tensor.matmul(out=p0[:, :HID], lhsT=xtt[:], rhs=winsb[:],
                                 start=True, stop=True)
                dg = sb.tile([128, HID], BF16, tag="dg")
                nc.sync.dma_start(dg[:], t_dege[nt * 128:(nt + 1) * 128, :])
                nc.vector.tensor_tensor(out=h_res[0][:, nt * HID:(nt + 1) * HID],
                                        in0=p0[:, :HID], in1=dg[:], op=OP.add)

            for l in range(L):
                hc = h_res[l % 2]
                hn_res = h_res[(l + 1) % 2]
                attr = cpool.tile([128, HID], BF16, tag=f"attr{l}")
                nc.sync.dma_start(attr[:], t_att[l])
                gamr = cpool.tile([128, HID], BF16, tag=f"gamr{l}")
                nc.sync.dma_start(gamr[:], t_gam[l])
                betr = cpool.tile([128, HID], BF16, tag=f"betr{l}")
                nc.sync.dma_start(betr[:], t_bet[l])
                bcvr = cpool.tile([128, HID], BF16, tag=f"bcvr{l}")
                nc.sync.dma_start(bcvr[:], t_bcv[l])
                wlr = cpool.tile([128, 2, 2 * HID], BF16, tag=f"wlr{l}")
                for half in range(2):
                    nc.sync.dma_start(wlr[:, half, :HID],
                                      t_Wl[l, half * 128:(half + 1) * 128, :])
                    nc.sync.dma_start(wlr[:, half, HID:],
                                      t_Wr[l, half * 128:(half + 1) * 128, :])

                # ---- B1: xl/xr for own shard; AllGather per chunk as soon
                # as its blocks are done so the collective overlaps B1 ----
                off = 0
                for nt in range(NB):
                    hT = sb.tile([128, HID], BF16, tag="hT")
                    for half in range(2):
                        tp = pt.tile([128, 128], BF16, tag="t128", space="PSUM")
                        nc.tensor.transpose(
                            out=tp[:],
                            in_=hc[:, nt * HID + half * 128:nt * HID + (half + 1) * 128],
                            identity=identb[:])
                        nc.scalar.activation(out=hT[:, half * 128:(half + 1) * 128],
                                             in_=tp[:], func=AF.Copy)
                    pxx = ps.tile([128, 2 * HID], F32, tag="mm", space="PSUM")
                    for half in range(2):
                        nc.tensor.matmul(out=pxx[:],
                                         lhsT=hT[:, half * 128:(half + 1) * 128],
                                         rhs=wlr[:, half, :],
                                         start=half == 0, stop=half == 1)
                    xl_o = sb.tile([128, HID], BF16, tag="xl_o")
                    nc.scalar.activation(out=xl_o[:], in_=pxx[:, :HID], func=AF.Copy)
                    nc.sync.dma_start(xl_sh[nt * 128:(nt + 1) * 128, :], xl_o[:])
                    nc.scalar.activation(out=xr_res[:, nt * HID:(nt + 1) * HID],
                                         in_=pxx[:, HID:], func=AF.Copy)
                    if nt + 1 in CHB:
                        ci = CHB.index(nt + 1) - 1
                        r0, r1 = CHB[ci] * 128, CHB[ci + 1] * 128
                        nrows = (r1 - r0) * NCORES
                        nc.gpsimd.collective_compute(
                            "AllGather", OP.bypass,
                            replica_groups=[list(range(NCORES))],
                            ins=[xl_sh[r0:r1, :].opt()],
                            outs=[xl_full[off:off + nrows, :].opt()])
                        off += nrows

                # ---- B3: edge phase, 2-stage software pipeline: emit
                # stage1(j+1) (gather + zt matmuls + Prelu) before stage2(j)
                # (wat/reduce/exp/wm/acc) so the next tile's zt matmuls are
                # not queued behind the acc matmul that waits on wm ----
                def stage1(j):
                    b, t = j // KT, j % KT
                    if j % GBM == 0:
                        mn = min(GBM, NT - j)
                        mbT = mp.tile([128, GBM * 128], BF16, tag="mbT")
                        nc.sync.dma_start(mbT[:, :mn * 128],
                                          t_mskT[:, j * 128:(j + mn) * 128])
                        _build.mbT = mbT
                    xe = gp.tile([128, HID], BF16, tag="xe")
                    nc.gpsimd.indirect_dma_start(
                        out=xe[:], out_offset=None, in_=xl_full[:],
                        in_offset=bass.IndirectOffsetOnAxis(
                            ap=sidx[:, j:j + 1], axis=0))
                    # z = xr[dl] + xl[src] assembled in PSUM
                    zt = pz.tile([128, HID], F32, tag="mm", space="PSUM")
                    nc.tensor.matmul(out=zt[:],
                                     lhsT=_build.mbT[:, (j % GBM) * 128:
                                                     (j % GBM + 1) * 128],
                                     rhs=xr_res[:, b * HID:(b + 1) * HID],
                                     start=True, stop=False)
                    nc.tensor.matmul(out=zt[:], lhsT=identb[:], rhs=xe[:],
                                     start=False, stop=True)
                    zl = sb.tile([128, HID], BF16, tag="zl")
                    nc.scalar.activation(out=zl[:], in_=zt[:], func=AF.Prelu,
                                         alpha=NEG)
                    return zt, zl

                def stage2(j, zt, zl):
                    b, t = j // KT, j % KT
                    wat = sb.tile([128, HID], BF16, tag="wat")
                    nc.vector.tensor_tensor(out=wat[:], in0=zl[:], in1=attr[:],
                                            op=OP.mult)
                    lg = sb.tile([128, HEADS], F32, tag="lg")
                    nc.vector.tensor_reduce(
                        out=lg[:], in_=wat[:].rearrange("p (h d) -> p h d", h=HEADS),
                        axis=mybir.AxisListType.X, op=OP.add)
                    wm = sb.tile([128, HID + HEADS], BF16, tag="wm")
                    nc.scalar.activation(out=wm[:, HID:], in_=lg[:], func=AF.Exp)
                    nc.vector.tensor_tensor(
                        out=wm[:, :HID].rearrange("p (h d) -> p h d", h=HEADS),
                        in0=zt[:].rearrange("p (h d) -> p h d", h=HEADS),
                        in1=wm[:, HID:].broadcast_to([128, HEADS, HD]),
                        op=OP.mult)
                    if j % GBM == 0:
                        mn = min(GBM, NT - j)
                        mb = mp.tile([128, GBM * 128], BF16, tag="mb")
                        nc.sync.dma_start(mb[:, :mn * 128],
                                          t_msk[:, j * 128:(j + mn) * 128])
                        _build.mb = mb
                    if t == 0:
                        acc = pacc.tile([128, HID + HEADS], F32, tag="acc",
                                        space="PSUM")
                        _build.acc = acc
                    nc.tensor.matmul(out=_build.acc[:],
                                     lhsT=_build.mb[:, (j % GBM) * 128:
                                                    (j % GBM + 1) * 128],
                                     rhs=wm[:], start=t == 0, stop=t == KT - 1)
                    return t == KT - 1

                pend = stage1(0)
                for j in range(NT):
                    nxt = stage1(j + 1) if j + 1 < NT else None
                    done = stage2(j, *pend)
                    pend = nxt
                    if not done:
                        continue
                    b, t = j // KT, j % KT
                    # ---- finalize block b ----
                    acc = _build.acc
                    rcp = sb.tile([128, HEADS], F32, tag="rcp")
                    nc.vector.reciprocal(rcp[:], acc[:, HID:])
                    v = sb.tile([128, HID], F32, tag="v")
                    nc.vector.tensor_tensor(
                        out=v[:].rearrange("p (h d) -> p h d", h=HEADS),
                        in0=acc[:, :HID].rearrange("p (h d) -> p h d", h=HEADS),
                        in1=rcp[:].broadcast_to([128, HEADS, HD]), op=OP.mult)
                    vx = sb.tile([128, HID], F32, tag="vx")
                    nc.vector.tensor_tensor(out=vx[:], in0=v[:],
                                            in1=xr_res[:, b * HID:(b + 1) * HID],
                                            op=OP.subtract)
                    vb = sb.tile([128, HID], F32, tag="vb")
                    nc.vector.tensor_tensor(out=vb[:], in0=vx[:], in1=bcvr[:],
                                            op=OP.add)
                    vr = sb.tile([128, HID], F32, tag="vr")
                    nc.vector.scalar_tensor_tensor(
                        out=vr[:], in0=vb[:], scalar=0.0,
                        in1=hc[:, b * HID:(b + 1) * HID],
                        op0=OP.max, op1=OP.add)
                    st6 = sb.tile([128, 6], F32, tag="st6")
                    nc.vector.bn_stats(st6[:], vr[:])
                    mv = sb.tile([128, 2], F32, tag="mv")
                    nc.vector.bn_aggr(mv[:], st6[:])
                    lnv = sb.tile([128, 1], F32, tag="lnv")
                    nc.scalar.activation(out=lnv[:], in_=mv[:, 1:2], func=AF.Ln,
                                         bias=epsc[:, :1])
                    rstd = sb.tile([128, 1], F32, tag="rstd")
                    nc.scalar.activation(out=rstd[:], in_=lnv[:], func=AF.Exp,
                                         scale=-0.5)
                    hng = sb.tile([128, HID], F32, tag="hng")
                    nc.vector.scalar_tensor_tensor(
                        out=hng[:], in0=vr[:], scalar=mv[:, 0:1], in1=gamr[:],
                        op0=OP.subtract, op1=OP.mult)
                    nc.vector.scalar_tensor_tensor(
                        out=hn_res[:, b * HID:(b + 1) * HID], in0=hng[:],
                        scalar=rstd[:, :1], in1=betr[:],
                        op0=OP.mult, op1=OP.add)
                    if l == L - 1:
                        # pool transposed: poolT[f, g] += hn[n, f]^T @ poh[n, g]
                        poh = sb.tile([128, G], BF16, tag="poh")
                        nc.sync.dma_start(poh[:], t_poh[b])
                        if b == 0:
                            ppA = ppool.tile([128, G], F32, tag="ppA",
                                             space="PSUM")
                            ppB = ppool.tile([128, G], F32, tag="ppB",
                                             space="PSUM")
                            _build.pp = (ppA, ppB)
                        for half in range(2):
                            nc.tensor.matmul(
                                out=_build.pp[half][:],
                                lhsT=hn_res[:, b * HID + half * 128:
                                            b * HID + (half + 1) * 128],
                                rhs=poh[:], start=b == 0, stop=b == NB - 1)

            # ---- stage C: pool (transposed layout) + classifier ----
            pool_sb = sb.tile([128, 2 * G], F32, tag="pool_sb")
            nc.scalar.activation(out=pool_sb[:, :G], in_=_build.pp[0][:], func=AF.Copy)
            nc.scalar.activation(out=pool_sb[:, G:], in_=_build.pp[1][:], func=AF.Copy)
            pl_in = dram.tile([128, 2 * G], F32)
            pl_out = dram.tile([128, 2 * G], F32)
            nc.sync.dma_start(pl_in[:], pool_sb[:])
            nc.gpsimd.collective_compute(
                "AllReduce", OP.add, replica_groups=[list(range(NCORES))],
                ins=[pl_in[:].opt()], outs=[pl_out[:].opt()])
            poolc = sb.tile([128, 2 * G], F32, tag="poolc")
            nc.sync.dma_start(poolc[:], pl_out[:])
            wc_sb = sb.tile([128, 2 * NCLS], F32, tag="wc")
            for half in range(2):
                nc.sync.dma_start(wc_sb[:, half * NCLS:(half + 1) * NCLS],
                                  t_Wc[half * 128:(half + 1) * 128, :])
            pcls = ppool.tile([G, NCLS], F32, tag="ppA", space="PSUM")
            for half in range(2):
                nc.tensor.matmul(out=pcls[:],
                                 lhsT=poolc[:, half * G:(half + 1) * G],
                                 rhs=wc_sb[:, half * NCLS:(half + 1) * NCLS],
                                 start=half == 0, stop=half == 1)
            bc_sb = sb.tile([G, NCLS], F32, tag="bc")
            nc.sync.dma_start(bc_sb[:], t_bc)
            res = sb.tile([G, NCLS], F32, tag="resout")
            nc.vector.tensor_tensor(out=res[:], in0=pcls[:], in1=bc_sb[:], op=OP.add)
            nc.sync.dma_start(t_out, res[:])

    nc.compile()
    return nc
